# revision 16
# baseline (speedup 1.0000x reference)
"""Trainium2 Bass kernel for nn_MicroSpeech: 2-layer diagonal complex LRU net.

Math: |lam| = exp(-exp(nu)) ~= 0.368 for nu ~ U[0, 0.01), so the recurrence
h_t = lam*h_{t-1} + u_t is an 8-tap FIR to ~2e-3, factorized radix-(4,2):
    h_t = sum_{j=0..1} lam^{4j} (sum_{k=0..3} lam^k u_{t-4j-k})
selu is decomposed with a CENTERED exp branch,
    selu(v) = L*relu(v) + L*A*e'(v),   e'(v) = min(exp(v), 1) - 1,
which leaves no additive constants anywhere (mlp bias handled via activation
bias), so zero-padded halos are self-consistent and no y_0 folding is needed.

Layout: each core's 8192 frames split into two 4096-frame halves, stacked on
SBUF partitions (half A in partitions 0:64, half B in 64:128 for all 64-dim
signals). Every matmul then contracts K=128 with bf16 operands (1 cy/col, and
the full-array activity keeps the PE HAM un-throttled at 2.4 GHz). PSUM
evacuations are spread across Vector, Scalar and GpSimd engines.

Sharding: data-parallel, frames split 8192/core across 8 cores with a 32-frame
input halo (no inter-core communication).
"""
import os

os.environ.setdefault("MYCRO_LOCAL_CACHE", "1")

import numpy as np
import ml_dtypes

BF16 = ml_dtypes.bfloat16

WINDOW = 128
H = 32
O2 = 256
L_TOTAL = 65536
NCORES = 8
F = L_TOTAL // NCORES          # frames per core
FH = F // 2                    # frames per half-sequence
HALO = 32
NIN = 480                      # interior frames per tile per half
NT = (FH + NIN - 1) // NIN     # 9 tiles
PAD_H = NIN * (NT - 1) + 512   # 4352 padded frames per half

SELU_L = 1.0507009873554805
SELU_A = 1.6732632423543772

CHUNK = int(os.environ.get("MICROSPEECH_CHUNK", "2"))
BSHIFTS = int(os.environ.get("MICROSPEECH_BSHIFTS", "2"))
MIN_ENGINE = os.environ.get("MICROSPEECH_MIN_ENGINE", "vector")
OUT_BF16 = os.environ.get("MICROSPEECH_OUT_BF16", "0") == "1"


# ---------------------------------------------------------------- host precompute
def _build_consts(inp):
    def Trep(mu):
        a, b = np.diag(mu.real), np.diag(mu.imag)
        return np.block([[a, -b], [b, a]])

    def layer(br, bi, nu, th):
        br, bi, nu, th = [np.asarray(a, np.float64) for a in (br, bi, nu, th)]
        lam = np.exp(-np.exp(nu) + 1j * np.exp(th))
        gamma = np.sqrt(1.0 - np.abs(lam) ** 2)
        B = (br + 1j * bi) * gamma[:, None]
        return lam, B

    def Eproj(C, mu):
        Cr, Ci = C.real, C.imag
        return np.hstack([Cr * mu.real[None, :] - Ci * mu.imag[None, :],
                          -Cr * mu.imag[None, :] - Ci * mu.real[None, :]])

    def bd(M):
        """blockdiag(M, M) for the two stacked sequence halves."""
        Z = np.zeros_like(M)
        return np.block([[M, Z], [Z, M]])

    lam1, B1 = layer(inp["b1r"], inp["b1i"], inp["nu1"], inp["th1"])
    lam2, B2 = layer(inp["b2r"], inp["b2i"], inp["nu2"], inp["th2"])
    C1 = np.asarray(inp["c1r"], np.float64) + 1j * np.asarray(inp["c1i"], np.float64)
    C2 = np.asarray(inp["c2r"], np.float64) + 1j * np.asarray(inp["c2i"], np.float64)
    D1 = np.asarray(inp["d1"], np.float64)
    D2 = np.asarray(inp["d2"], np.float64)
    W = np.asarray(inp["mlp_w"], np.float64)

    o = {}
    o["lhsT_u1"] = np.vstack([B1.real, B1.imag]).T                      # (128, 64)
    for k in range(4):
        o[f"lhsT_A1_{k}"] = bd(Trep(lam1 ** k)).T                       # (128, 128)
    for j in range(BSHIFTS):
        o[f"lhsT_B1_{j}"] = bd(Eproj(C1, lam1 ** (4 * j))).T            # (128, 64)
    o["lhsT_D1"] = D1.T                                                 # (128, 32)

    # CE1 rows = [c1_A; c1_B; e1_A; e1_B]; cols = [z_A; z_B]
    m = np.zeros((128, 64))
    m[0:32, 0:32] = SELU_L * W
    m[32:64, 32:64] = SELU_L * W
    m[64:96, 0:32] = SELU_L * SELU_A * W
    m[96:128, 32:64] = SELU_L * SELU_A * W
    o["lhsT_mlp"] = m

    B2s = np.vstack([B2.real, B2.imag])                                 # (64, 32)
    o["lhsT_u2"] = np.vstack([SELU_L * B2s.T, SELU_L * SELU_A * B2s.T])  # (64, 64)
    for k in range(4):
        o[f"lhsT_A2_{k}"] = bd(Trep(lam2 ** k)).T                       # (128, 128)
    for j in range(BSHIFTS):
        o[f"lhsT_B2_{j}"] = bd(Trep(lam2 ** (4 * j))).T                 # (128, 128)
    G = np.hstack([SELU_L * D2, SELU_L * SELU_A * D2, C2.real, -C2.imag])
    o["lhsT_P2a"] = G[:128].T                                           # (128, 128)
    o["lhsT_P2b"] = G[128:].T
    return {k: np.asarray(v) for k, v in o.items()}


_BLOB_SPECS = (
    [("ident", 128), ("lhsT_u1", 64)]
    + [(f"lhsT_A1_{k}", 128) for k in range(4)]
    + [(f"lhsT_B1_{j}", 64) for j in range(BSHIFTS)]
    + [("lhsT_D1", 32), ("lhsT_mlp", 64), ("lhsT_u2", 64)]
    + [(f"lhsT_A2_{k}", 128) for k in range(4)]
    + [(f"lhsT_B2_{j}", 128) for j in range(BSHIFTS)]
    + [("lhsT_P2a", 128), ("lhsT_P2b", 128)]
)
_BLOB_OFF = {}
_c = 0
for _n, _w in _BLOB_SPECS:
    _BLOB_OFF[_n] = _c
    _c += _w
BLOB_COLS = _c


def _pack_blob(consts):
    blob = np.zeros((128, BLOB_COLS), np.float32)
    blob[:, :128] = np.eye(128, dtype=np.float32)
    for name, wdt in _BLOB_SPECS:
        if name == "ident":
            continue
        m = consts[name].astype(np.float32)
        off = _BLOB_OFF[name]
        blob[: m.shape[0], off: off + m.shape[1]] = m
    return blob.astype(BF16)


# ---------------------------------------------------------------- bass program
_PROGRAM = None


def _build_program():
    import concourse.bacc as bacc
    import concourse.tile as tile
    from concourse import mybir

    nc = bacc.Bacc(None, target_bir_lowering=False)
    dt = mybir.dt
    AF = mybir.ActivationFunctionType
    ALU = mybir.AluOpType

    xin = nc.declare_dram_parameter("xin", [2, PAD_H, WINDOW], dt.bfloat16,
                                    isOutput=False)
    wts_d = nc.declare_dram_parameter("wts", [128, BLOB_COLS], dt.bfloat16,
                                      isOutput=False)
    bias_d = nc.declare_dram_parameter("bias", [64, 1], dt.float32, isOutput=False)
    out_dt = dt.bfloat16 if OUT_BF16 else dt.float32
    yout = nc.declare_dram_parameter("yout", [O2, F], out_dt, isOutput=True)

    def W(name, p=128):
        off = _BLOB_OFF[name]
        wdt = dict(_BLOB_SPECS)[name]
        return wts[:p, off: off + wdt]

    with tile.TileContext(nc) as tc:
        with (
            tc.tile_pool(name="singles", bufs=1) as singles,
            tc.tile_pool(name="work", bufs=4) as work,
            tc.tile_pool(name="psum", bufs=8, space="PSUM") as psum,
        ):
            wts = singles.tile([128, BLOB_COLS], dt.bfloat16)
            nc.sync.dma_start(out=wts, in_=wts_d[:, :])
            bias64 = singles.tile([64, 1], dt.float32)
            nc.sync.dma_start(out=bias64, in_=bias_d[:, :])

            def mm(out, lhsT, rhs, start, stop):
                nc.tensor.matmul(out, lhsT, rhs, start=start, stop=stop)

            mineng = {"gpsimd": nc.gpsimd, "vector": nc.vector}[MIN_ENGINE]

            for ch in range(0, NT, CHUNK):
                tt = list(range(ch, min(ch + CHUNK, NT)))
                nint = {t: min(NIN, FH - NIN * t) for t in tt}
                s4, xsb, u1sb, p1sb, CE1, ZP, u2sb, p2sb = ({} for _ in range(8))

                # ---- load both halves' x windows (frame-major)
                for t in tt:
                    f0 = NIN * t
                    s4[t] = {}
                    for hx in (0, 1):
                        s = work.tile([128, 512], dt.bfloat16, tag=f"s4{hx}",
                                      name=f"s4_{hx}_{t}")
                        nc.sync.dma_start(
                            out=s.rearrange("p (b w) -> p b w", b=4),
                            in_=xin[hx, f0: f0 + 512, :]
                            .rearrange("(b p) w -> p b w", p=128))
                        s4[t][hx] = s

                # ---- transpose to sample-major xsb (128 samples, 512 frames)
                for t in tt:
                    xsb[t] = {}
                    for hx in (0, 1):
                        xT = psum.tile([128, 512], dt.bfloat16, tag="ps")
                        for bb in range(4):
                            nc.tensor.transpose(
                                xT[:, bb * 128:(bb + 1) * 128],
                                s4[t][hx][:, bb * 128:(bb + 1) * 128],
                                wts[:, 0:128])
                        xsb[t][hx] = work.tile([128, 512], dt.bfloat16,
                                               tag=f"xsb{hx}", name=f"xsb_{hx}_{t}")
                        nc.vector.tensor_copy(out=xsb[t][hx], in_=xT)

                # ---- u1 = B~1 @ x, both halves stacked, frames [0,512)
                for t in tt:
                    u1ps = psum.tile([128, 512], dt.float32, tag="ps")
                    mm(u1ps[0:64, :], W("lhsT_u1"), xsb[t][0], True, True)
                    mm(u1ps[64:128, :], W("lhsT_u1"), xsb[t][1], True, True)
                    u1sb[t] = work.tile([128, 512], dt.bfloat16, tag="u1sb",
                                        name=f"u1sb{t}")
                    nc.vector.tensor_copy(out=u1sb[t], in_=u1ps)

                # ---- stage A1: p1[c] = sum_k T1_k u1[c-k], c in [4,512)
                for t in tt:
                    p1ps = psum.tile([128, 508], dt.float32, tag="ps")
                    for k in range(4):
                        mm(p1ps, W(f"lhsT_A1_{k}"),
                           u1sb[t][:, 4 - k:512 - k], k == 0, k == 3)
                    p1sb[t] = work.tile([128, 512], dt.bfloat16, tag="p1sb",
                                        name=f"p1sb{t}")
                    nc.vector.tensor_copy(out=p1sb[t][:, 4:512], in_=p1ps)

                # ---- stage B1 + D1 -> y1 [16,512); selu1 -> CE1
                for t in tt:
                    y1ps = psum.tile([64, 496], dt.float32, tag="ps")
                    for j in range(BSHIFTS):
                        mm(y1ps, W(f"lhsT_B1_{j}"),
                           p1sb[t][:, 16 - 4 * j:512 - 4 * j], j == 0, False)
                    mm(y1ps[0:32, :], W("lhsT_D1"), xsb[t][0][:, 16:512],
                       False, False)
                    mm(y1ps[32:64, :], W("lhsT_D1"), xsb[t][1][:, 16:512],
                       False, True)
                    CE1[t] = work.tile([128, 512], dt.bfloat16, tag="CE1",
                                       name=f"CE1_{t}")
                    nc.scalar.activation(out=CE1[t][0:64, 16:512], in_=y1ps,
                                         func=AF.Relu)
                    E1 = work.tile([64, 512], dt.float32, tag="E1")
                    nc.scalar.activation(out=E1[:, 16:512], in_=y1ps, func=AF.Exp)
                    mineng.tensor_scalar(
                        out=CE1[t][64:128, 16:512], in0=E1[:, 16:512],
                        scalar1=1.0, scalar2=-1.0, op0=ALU.min, op1=ALU.add)

                # ---- mlp -> z [16,512); selu2 -> ZP cols (A: 0:512, B: 512:1024)
                for t in tt:
                    zps = psum.tile([64, 496], dt.float32, tag="ps")
                    mm(zps, W("lhsT_mlp"), CE1[t][:, 16:512], True, True)
                    ZP[t] = work.tile([128, 1024], dt.bfloat16, tag="ZP",
                                      name=f"ZP_{t}")
                    nc.scalar.activation(out=ZP[t][0:32, 16:512], in_=zps[0:32, :],
                                         func=AF.Relu, bias=bias64[0:32, 0:1])
                    nc.scalar.activation(out=ZP[t][0:32, 528:1024],
                                         in_=zps[32:64, :], func=AF.Relu,
                                         bias=bias64[32:64, 0:1])
                    E2 = work.tile([64, 512], dt.float32, tag="E2")
                    nc.scalar.activation(out=E2[:, 16:512], in_=zps, func=AF.Exp,
                                         bias=bias64[:, 0:1])
                    mineng.tensor_scalar(
                        out=ZP[t][32:64, 16:512], in0=E2[0:32, 16:512],
                        scalar1=1.0, scalar2=-1.0, op0=ALU.min, op1=ALU.add)
                    mineng.tensor_scalar(
                        out=ZP[t][32:64, 528:1024], in0=E2[32:64, 16:512],
                        scalar1=1.0, scalar2=-1.0, op0=ALU.min, op1=ALU.add)

                # ---- u2 [16,512), both halves stacked
                for t in tt:
                    u2ps = psum.tile([128, 496], dt.float32, tag="ps")
                    mm(u2ps[0:64, :], W("lhsT_u2", p=64), ZP[t][0:64, 16:512],
                       True, True)
                    mm(u2ps[64:128, :], W("lhsT_u2", p=64), ZP[t][0:64, 528:1024],
                       True, True)
                    u2sb[t] = work.tile([128, 512], dt.bfloat16, tag="u2sb",
                                        name=f"u2sb{t}")
                    nc.vector.tensor_copy(out=u2sb[t][:, 16:512], in_=u2ps)

                # ---- stage A2: p2 [20,512)
                for t in tt:
                    p2ps = psum.tile([128, 492], dt.float32, tag="ps")
                    for k in range(4):
                        mm(p2ps, W(f"lhsT_A2_{k}"),
                           u2sb[t][:, 20 - k:512 - k], k == 0, k == 3)
                    p2sb[t] = work.tile([128, 512], dt.bfloat16, tag="p2sb",
                                        name=f"p2sb{t}")
                    nc.vector.tensor_copy(out=p2sb[t][:, 20:512], in_=p2ps)

                # ---- stage B2 -> h2 [32, 32+n) -> ZP rows 64:128
                for t in tt:
                    n = nint[t]
                    h2ps = psum.tile([128, 480], dt.float32, tag="ps")
                    for j in range(BSHIFTS):
                        mm(h2ps[:, :n], W(f"lhsT_B2_{j}"),
                           p2sb[t][:, 32 - 4 * j:32 - 4 * j + n],
                           j == 0, j == BSHIFTS - 1)
                    nc.scalar.activation(out=ZP[t][64:128, 32:32 + n],
                                         in_=h2ps[0:64, :n], func=AF.Copy)
                    nc.scalar.activation(out=ZP[t][64:128, 544:544 + n],
                                         in_=h2ps[64:128, :n], func=AF.Copy)

                # ---- projection + store (per half, per output row-block)
                for t in tt:
                    n = nint[t]
                    for hx in (0, 1):
                        c0 = FH * hx + NIN * t
                        zcols = (32, 544)[hx]
                        for half, ev in ((0, "v" if hx == 0 else "s"), (1, "s")):
                            yps = psum.tile([128, 480], dt.float32, tag="ps")
                            mm(yps[:, :n],
                               W("lhsT_P2a" if half == 0 else "lhsT_P2b"),
                               ZP[t][:, zcols:zcols + n], True, True)
                            yo = work.tile([128, 480], out_dt, tag=f"yo{hx}{half}")
                            if ev == "v":
                                nc.vector.tensor_copy(out=yo[:, :n],
                                                      in_=yps[:, :n])
                            else:
                                nc.scalar.activation(out=yo[:, :n],
                                                     in_=yps[:, :n], func=AF.Copy)
                            nc.sync.dma_start(
                                out=yout[half * 128:(half + 1) * 128, c0:c0 + n],
                                in_=yo[:, :n])
    nc.finalize()
    return nc


def _get_program():
    global _PROGRAM
    if _PROGRAM is None:
        _PROGRAM = _build_program()
    return _PROGRAM


# ---------------------------------------------------------------- host wrapper
def _make_inmaps(inputs):
    consts = _build_consts(inputs)
    blob = _pack_blob(consts)
    b = np.asarray(inputs["mlp_b"], np.float32)
    bias = np.concatenate([b, b]).reshape(64, 1).astype(np.float32)
    ts = np.asarray(inputs["inputs_timeseries"], np.float32).ravel()
    in_maps = []
    for core in range(NCORES):
        xpad = np.zeros((2, PAD_H * WINDOW), np.float32)
        for hx in (0, 1):
            s0 = core * F + hx * FH
            g0 = (s0 - HALO) * WINDOW
            g1 = min((s0 - HALO + PAD_H) * WINDOW, ts.size)
            a0 = max(0, -g0)
            xpad[hx, a0: a0 + (g1 - max(g0, 0))] = ts[max(g0, 0): g1]
        in_maps.append({
            "xin": xpad.reshape(2, PAD_H, WINDOW).astype(BF16),
            "wts": blob,
            "bias": bias,
        })
    return in_maps


def _enable_axon_trace():
    """Shim the missing antenv.axon_hooks so trace=True works under axon."""
    import sys
    import types

    if "antenv.axon_hooks" not in sys.modules:
        from trn_agent_boot.trn_boot import _ntff_profile_via_ctypes

        mod = types.ModuleType("antenv.axon_hooks")
        state = {"hook": None}
        mod.set_axon_ntff_profile_hook = lambda h: state.__setitem__("hook", h)
        mod.get_axon_ntff_profile_hook = lambda: state["hook"]
        sys.modules["antenv.axon_hooks"] = mod
        try:
            import antenv

            antenv.axon_hooks = mod
        except ImportError:
            pass
        hook = _ntff_profile_via_ctypes("/opt/axon/libaxon_pjrt.so")
        assert hook is not None
        mod.set_axon_ntff_profile_hook(hook)
    import concourse.bass_utils as bu

    bu.upload_artifacts = lambda tmpdir: tmpdir


def run(inputs, trace=False, **trace_kwargs):
    from concourse.bass_utils import run_bass_kernel_spmd

    if trace:
        _enable_axon_trace()
    nc = _get_program()
    in_maps = _make_inmaps(inputs)
    res = run_bass_kernel_spmd(nc, in_maps, list(range(NCORES)), trace=trace,
                               **trace_kwargs)
    out = np.concatenate(
        [np.asarray(r["yout"]).astype(np.float32) for r in res.results], axis=1)
    return out, res


def kernel(**inputs) -> np.ndarray:
    out, _ = run(inputs)
    return out


# revision 19
# speedup vs baseline: 1.1294x; 1.1294x over previous
"""Trainium2 Bass kernel for nn_MicroSpeech: 2-layer diagonal complex LRU net.

Math: |lam| = exp(-exp(nu)) ~= 0.368 for nu ~ U[0, 0.01), so the recurrence
h_t = lam*h_{t-1} + u_t is an 8-tap FIR to ~2e-3, factorized radix-(4,2):
    h_t = sum_{j=0..1} lam^{4j} (sum_{k=0..3} lam^k u_{t-4j-k})
selu is decomposed with a CENTERED exp branch,
    selu(v) = L*relu(v) + L*A*e'(v),   e'(v) = min(exp(v), 1) - 1,
which leaves no additive constants anywhere (mlp bias handled via activation
bias), so zero-padded halos are self-consistent and no y_0 folding is needed.

Layout: each core's 8192 frames split into two 4096-frame halves, stacked on
SBUF partitions (half A in partitions 0:64, half B in 64:128 for all 64-dim
signals). Every matmul then contracts K=128 with bf16 operands (1 cy/col, and
the full-array activity keeps the PE HAM un-throttled at 2.4 GHz). PSUM
evacuations are spread across Vector, Scalar and GpSimd engines.

Sharding: data-parallel, frames split 8192/core across 8 cores with a 32-frame
input halo (no inter-core communication).
"""
import os

os.environ.setdefault("MYCRO_LOCAL_CACHE", "1")

import numpy as np
import ml_dtypes

BF16 = ml_dtypes.bfloat16

WINDOW = 128
H = 32
O2 = 256
L_TOTAL = 65536
NCORES = 8
F = L_TOTAL // NCORES          # frames per core
FH = F // 2                    # frames per half-sequence
HALO = 32
NIN = 480                      # interior frames per tile per half
NT = (FH + NIN - 1) // NIN     # 9 tiles
PAD_H = NIN * (NT - 1) + 512   # 4352 padded frames per half

SELU_L = 1.0507009873554805
SELU_A = 1.6732632423543772

CHUNK = int(os.environ.get("MICROSPEECH_CHUNK", "3"))
BSHIFTS = int(os.environ.get("MICROSPEECH_BSHIFTS", "2"))
DMA_T = os.environ.get("MICROSPEECH_DMA_T", "1") == "1"
MIN_ENGINE = os.environ.get("MICROSPEECH_MIN_ENGINE", "vector")
OUT_BF16 = os.environ.get("MICROSPEECH_OUT_BF16", "0") == "1"


# ---------------------------------------------------------------- host precompute
def _build_consts(inp):
    def Trep(mu):
        a, b = np.diag(mu.real), np.diag(mu.imag)
        return np.block([[a, -b], [b, a]])

    def layer(br, bi, nu, th):
        br, bi, nu, th = [np.asarray(a, np.float64) for a in (br, bi, nu, th)]
        lam = np.exp(-np.exp(nu) + 1j * np.exp(th))
        gamma = np.sqrt(1.0 - np.abs(lam) ** 2)
        B = (br + 1j * bi) * gamma[:, None]
        return lam, B

    def Eproj(C, mu):
        Cr, Ci = C.real, C.imag
        return np.hstack([Cr * mu.real[None, :] - Ci * mu.imag[None, :],
                          -Cr * mu.imag[None, :] - Ci * mu.real[None, :]])

    def bd(M):
        """blockdiag(M, M) for the two stacked sequence halves."""
        Z = np.zeros_like(M)
        return np.block([[M, Z], [Z, M]])

    lam1, B1 = layer(inp["b1r"], inp["b1i"], inp["nu1"], inp["th1"])
    lam2, B2 = layer(inp["b2r"], inp["b2i"], inp["nu2"], inp["th2"])
    C1 = np.asarray(inp["c1r"], np.float64) + 1j * np.asarray(inp["c1i"], np.float64)
    C2 = np.asarray(inp["c2r"], np.float64) + 1j * np.asarray(inp["c2i"], np.float64)
    D1 = np.asarray(inp["d1"], np.float64)
    D2 = np.asarray(inp["d2"], np.float64)
    W = np.asarray(inp["mlp_w"], np.float64)

    o = {}
    o["lhsT_u1"] = np.vstack([B1.real, B1.imag]).T                      # (128, 64)
    for k in range(4):
        o[f"lhsT_A1_{k}"] = bd(Trep(lam1 ** k)).T                       # (128, 128)
    for j in range(BSHIFTS):
        o[f"lhsT_B1_{j}"] = bd(Eproj(C1, lam1 ** (4 * j))).T            # (128, 64)
    o["lhsT_D1"] = D1.T                                                 # (128, 32)

    # CE1 rows = [c1_A; c1_B; e1_A; e1_B]; cols = [z_A; z_B]
    m = np.zeros((128, 64))
    m[0:32, 0:32] = SELU_L * W
    m[32:64, 32:64] = SELU_L * W
    m[64:96, 0:32] = SELU_L * SELU_A * W
    m[96:128, 32:64] = SELU_L * SELU_A * W
    o["lhsT_mlp"] = m

    B2s = np.vstack([B2.real, B2.imag])                                 # (64, 32)
    o["lhsT_u2"] = np.vstack([SELU_L * B2s.T, SELU_L * SELU_A * B2s.T])  # (64, 64)
    for k in range(4):
        o[f"lhsT_A2_{k}"] = bd(Trep(lam2 ** k)).T                       # (128, 128)
    for j in range(BSHIFTS):
        o[f"lhsT_B2_{j}"] = bd(Trep(lam2 ** (4 * j))).T                 # (128, 128)
    G = np.hstack([SELU_L * D2, SELU_L * SELU_A * D2, C2.real, -C2.imag])
    o["lhsT_P2a"] = G[:128].T                                           # (128, 128)
    o["lhsT_P2b"] = G[128:].T
    return {k: np.asarray(v) for k, v in o.items()}


_BLOB_SPECS = (
    [("ident", 128), ("lhsT_u1", 64)]
    + [(f"lhsT_A1_{k}", 128) for k in range(4)]
    + [(f"lhsT_B1_{j}", 64) for j in range(BSHIFTS)]
    + [("lhsT_D1", 32), ("lhsT_mlp", 64), ("lhsT_u2", 64)]
    + [(f"lhsT_A2_{k}", 128) for k in range(4)]
    + [(f"lhsT_B2_{j}", 128) for j in range(BSHIFTS)]
    + [("lhsT_P2a", 128), ("lhsT_P2b", 128)]
)
_BLOB_OFF = {}
_c = 0
for _n, _w in _BLOB_SPECS:
    _BLOB_OFF[_n] = _c
    _c += _w
BLOB_COLS = _c


def _pack_blob(consts):
    blob = np.zeros((128, BLOB_COLS), np.float32)
    blob[:, :128] = np.eye(128, dtype=np.float32)
    for name, wdt in _BLOB_SPECS:
        if name == "ident":
            continue
        m = consts[name].astype(np.float32)
        off = _BLOB_OFF[name]
        blob[: m.shape[0], off: off + m.shape[1]] = m
    return blob.astype(BF16)


# ---------------------------------------------------------------- bass program
_PROGRAM = None


def _build_program():
    import concourse.bacc as bacc
    import concourse.tile as tile
    from concourse import mybir

    nc = bacc.Bacc(None, target_bir_lowering=False)
    dt = mybir.dt
    AF = mybir.ActivationFunctionType
    ALU = mybir.AluOpType

    xin = nc.declare_dram_parameter("xin", [2, PAD_H, WINDOW], dt.bfloat16,
                                    isOutput=False)
    wts_d = nc.declare_dram_parameter("wts", [128, BLOB_COLS], dt.bfloat16,
                                      isOutput=False)
    bias_d = nc.declare_dram_parameter("bias", [64, 1], dt.float32, isOutput=False)
    out_dt = dt.bfloat16 if OUT_BF16 else dt.float32
    yout = nc.declare_dram_parameter("yout", [O2, F], out_dt, isOutput=True)

    def W(name, p=128):
        off = _BLOB_OFF[name]
        wdt = dict(_BLOB_SPECS)[name]
        return wts[:p, off: off + wdt]

    with tile.TileContext(nc) as tc:
        with (
            tc.tile_pool(name="singles", bufs=1) as singles,
            tc.tile_pool(name="work", bufs=4) as work,
            tc.tile_pool(name="psum", bufs=8, space="PSUM") as psum,
        ):
            wts = singles.tile([128, BLOB_COLS], dt.bfloat16)
            nc.sync.dma_start(out=wts, in_=wts_d[:, :])
            bias64 = singles.tile([64, 1], dt.float32)
            nc.sync.dma_start(out=bias64, in_=bias_d[:, :])

            def mm(out, lhsT, rhs, start, stop):
                nc.tensor.matmul(out, lhsT, rhs, start=start, stop=stop)

            mineng = {"gpsimd": nc.gpsimd, "vector": nc.vector}[MIN_ENGINE]

            for ch in range(0, NT, CHUNK):
                tt = list(range(ch, min(ch + CHUNK, NT)))
                nint = {t: min(NIN, FH - NIN * t) for t in tt}
                s4, xsb, u1sb, p1sb, CE1, ZP, u2sb, p2sb = ({} for _ in range(8))

                # ---- load x windows directly transposed (xbar DMA) or via PE
                if DMA_T:
                    for t in tt:
                        f0 = NIN * t
                        xsb[t] = {}
                        for hx in (0, 1):
                            xsb[t][hx] = work.tile(
                                [128, 512], dt.bfloat16, tag=f"xsb{hx}",
                                name=f"xsb_{hx}_{t}")
                            nc.sync.dma_start_transpose(
                                out=xsb[t][hx], in_=xin[hx, f0: f0 + 512, :])
                else:
                    for t in tt:
                        f0 = NIN * t
                        s4[t] = {}
                        for hx in (0, 1):
                            s = work.tile([128, 512], dt.bfloat16, tag=f"s4{hx}",
                                          name=f"s4_{hx}_{t}")
                            nc.sync.dma_start(
                                out=s.rearrange("p (b w) -> p b w", b=4),
                                in_=xin[hx, f0: f0 + 512, :]
                                .rearrange("(b p) w -> p b w", p=128))
                            s4[t][hx] = s
                    for t in tt:
                        xsb[t] = {}
                        for hx in (0, 1):
                            xT = psum.tile([128, 512], dt.bfloat16, tag="ps")
                            for bb in range(4):
                                nc.tensor.transpose(
                                    xT[:, bb * 128:(bb + 1) * 128],
                                    s4[t][hx][:, bb * 128:(bb + 1) * 128],
                                    wts[:, 0:128])
                            xsb[t][hx] = work.tile(
                                [128, 512], dt.bfloat16, tag=f"xsb{hx}",
                                name=f"xsb_{hx}_{t}")
                            nc.vector.tensor_copy(out=xsb[t][hx], in_=xT)

                # ---- u1 = B~1 @ x, both halves stacked, frames [0,512)
                for t in tt:
                    u1ps = psum.tile([128, 512], dt.float32, tag="ps")
                    mm(u1ps[0:64, :], W("lhsT_u1"), xsb[t][0], True, True)
                    mm(u1ps[64:128, :], W("lhsT_u1"), xsb[t][1], True, True)
                    u1sb[t] = work.tile([128, 512], dt.bfloat16, tag="u1sb",
                                        name=f"u1sb{t}")
                    nc.vector.tensor_copy(out=u1sb[t], in_=u1ps)

                # ---- stage A1: p1[c] = sum_k T1_k u1[c-k], c in [4,512)
                for t in tt:
                    p1ps = psum.tile([128, 508], dt.float32, tag="ps")
                    for k in range(4):
                        mm(p1ps, W(f"lhsT_A1_{k}"),
                           u1sb[t][:, 4 - k:512 - k], k == 0, k == 3)
                    p1sb[t] = work.tile([128, 512], dt.bfloat16, tag="p1sb",
                                        name=f"p1sb{t}")
                    nc.vector.tensor_copy(out=p1sb[t][:, 4:512], in_=p1ps)

                # ---- stage B1 + D1 -> y1 [16,512); selu1 -> CE1
                for t in tt:
                    y1ps = psum.tile([64, 496], dt.float32, tag="ps")
                    for j in range(BSHIFTS):
                        mm(y1ps, W(f"lhsT_B1_{j}"),
                           p1sb[t][:, 16 - 4 * j:512 - 4 * j], j == 0, False)
                    mm(y1ps[0:32, :], W("lhsT_D1"), xsb[t][0][:, 16:512],
                       False, False)
                    mm(y1ps[32:64, :], W("lhsT_D1"), xsb[t][1][:, 16:512],
                       False, True)
                    CE1[t] = work.tile([128, 512], dt.bfloat16, tag="CE1",
                                       name=f"CE1_{t}")
                    nc.scalar.activation(out=CE1[t][0:64, 16:512], in_=y1ps,
                                         func=AF.Relu)
                    E1 = work.tile([64, 512], dt.float32, tag="E1")
                    nc.scalar.activation(out=E1[:, 16:512], in_=y1ps, func=AF.Exp)
                    mineng.tensor_scalar(
                        out=CE1[t][64:128, 16:512], in0=E1[:, 16:512],
                        scalar1=1.0, scalar2=-1.0, op0=ALU.min, op1=ALU.add)

                # ---- mlp -> z [16,512); selu2 -> ZP cols (A: 0:512, B: 512:1024)
                for t in tt:
                    zps = psum.tile([64, 496], dt.float32, tag="ps")
                    mm(zps, W("lhsT_mlp"), CE1[t][:, 16:512], True, True)
                    ZP[t] = work.tile([128, 1024], dt.bfloat16, tag="ZP",
                                      name=f"ZP_{t}")
                    nc.scalar.activation(out=ZP[t][0:32, 16:512], in_=zps[0:32, :],
                                         func=AF.Relu, bias=bias64[0:32, 0:1])
                    nc.scalar.activation(out=ZP[t][0:32, 528:1024],
                                         in_=zps[32:64, :], func=AF.Relu,
                                         bias=bias64[32:64, 0:1])
                    E2 = work.tile([64, 512], dt.float32, tag="E2")
                    nc.scalar.activation(out=E2[:, 16:512], in_=zps, func=AF.Exp,
                                         bias=bias64[:, 0:1])
                    mineng.tensor_scalar(
                        out=ZP[t][32:64, 16:512], in0=E2[0:32, 16:512],
                        scalar1=1.0, scalar2=-1.0, op0=ALU.min, op1=ALU.add)
                    mineng.tensor_scalar(
                        out=ZP[t][32:64, 528:1024], in0=E2[32:64, 16:512],
                        scalar1=1.0, scalar2=-1.0, op0=ALU.min, op1=ALU.add)

                # ---- u2 [16,512), both halves stacked
                for t in tt:
                    u2ps = psum.tile([128, 496], dt.float32, tag="ps")
                    mm(u2ps[0:64, :], W("lhsT_u2", p=64), ZP[t][0:64, 16:512],
                       True, True)
                    mm(u2ps[64:128, :], W("lhsT_u2", p=64), ZP[t][0:64, 528:1024],
                       True, True)
                    u2sb[t] = work.tile([128, 512], dt.bfloat16, tag="u2sb",
                                        name=f"u2sb{t}")
                    nc.vector.tensor_copy(out=u2sb[t][:, 16:512], in_=u2ps)

                # ---- stage A2: p2 [20,512)
                for t in tt:
                    p2ps = psum.tile([128, 492], dt.float32, tag="ps")
                    for k in range(4):
                        mm(p2ps, W(f"lhsT_A2_{k}"),
                           u2sb[t][:, 20 - k:512 - k], k == 0, k == 3)
                    p2sb[t] = work.tile([128, 512], dt.bfloat16, tag="p2sb",
                                        name=f"p2sb{t}")
                    nc.vector.tensor_copy(out=p2sb[t][:, 20:512], in_=p2ps)

                # ---- stage B2 -> h2 [32, 32+n) -> ZP rows 64:128
                for t in tt:
                    n = nint[t]
                    h2ps = psum.tile([128, 480], dt.float32, tag="ps")
                    for j in range(BSHIFTS):
                        mm(h2ps[:, :n], W(f"lhsT_B2_{j}"),
                           p2sb[t][:, 32 - 4 * j:32 - 4 * j + n],
                           j == 0, j == BSHIFTS - 1)
                    nc.scalar.activation(out=ZP[t][64:128, 32:32 + n],
                                         in_=h2ps[0:64, :n], func=AF.Copy)
                    nc.scalar.activation(out=ZP[t][64:128, 544:544 + n],
                                         in_=h2ps[64:128, :n], func=AF.Copy)

                # ---- projection + store (per half, per output row-block)
                for t in tt:
                    n = nint[t]
                    for hx in (0, 1):
                        c0 = FH * hx + NIN * t
                        zcols = (32, 544)[hx]
                        for half, ev in ((0, "v"), (1, "s")):
                            yps = psum.tile([128, 480], dt.float32, tag="ps")
                            mm(yps[:, :n],
                               W("lhsT_P2a" if half == 0 else "lhsT_P2b"),
                               ZP[t][:, zcols:zcols + n], True, True)
                            yo = work.tile([128, 480], out_dt, tag=f"yo{hx}{half}")
                            if ev == "v":
                                nc.vector.tensor_copy(out=yo[:, :n],
                                                      in_=yps[:, :n])
                            else:
                                nc.scalar.activation(out=yo[:, :n],
                                                     in_=yps[:, :n], func=AF.Copy)
                            nc.sync.dma_start(
                                out=yout[half * 128:(half + 1) * 128, c0:c0 + n],
                                in_=yo[:, :n])
    nc.finalize()
    return nc


def _get_program():
    global _PROGRAM
    if _PROGRAM is None:
        _PROGRAM = _build_program()
    return _PROGRAM


# ---------------------------------------------------------------- host wrapper
def _make_inmaps(inputs):
    consts = _build_consts(inputs)
    blob = _pack_blob(consts)
    b = np.asarray(inputs["mlp_b"], np.float32)
    bias = np.concatenate([b, b]).reshape(64, 1).astype(np.float32)
    ts = np.asarray(inputs["inputs_timeseries"], np.float32).ravel()
    in_maps = []
    for core in range(NCORES):
        xpad = np.zeros((2, PAD_H * WINDOW), np.float32)
        for hx in (0, 1):
            s0 = core * F + hx * FH
            g0 = (s0 - HALO) * WINDOW
            g1 = min((s0 - HALO + PAD_H) * WINDOW, ts.size)
            a0 = max(0, -g0)
            xpad[hx, a0: a0 + (g1 - max(g0, 0))] = ts[max(g0, 0): g1]
        in_maps.append({
            "xin": xpad.reshape(2, PAD_H, WINDOW).astype(BF16),
            "wts": blob,
            "bias": bias,
        })
    return in_maps


def _enable_axon_trace():
    """Shim the missing antenv.axon_hooks so trace=True works under axon."""
    import sys
    import types

    if "antenv.axon_hooks" not in sys.modules:
        from trn_agent_boot.trn_boot import _ntff_profile_via_ctypes

        mod = types.ModuleType("antenv.axon_hooks")
        state = {"hook": None}
        mod.set_axon_ntff_profile_hook = lambda h: state.__setitem__("hook", h)
        mod.get_axon_ntff_profile_hook = lambda: state["hook"]
        sys.modules["antenv.axon_hooks"] = mod
        try:
            import antenv

            antenv.axon_hooks = mod
        except ImportError:
            pass
        hook = _ntff_profile_via_ctypes("/opt/axon/libaxon_pjrt.so")
        assert hook is not None
        mod.set_axon_ntff_profile_hook(hook)
    import concourse.bass_utils as bu

    bu.upload_artifacts = lambda tmpdir: tmpdir


def run(inputs, trace=False, **trace_kwargs):
    from concourse.bass_utils import run_bass_kernel_spmd

    if trace:
        _enable_axon_trace()
    nc = _get_program()
    in_maps = _make_inmaps(inputs)
    res = run_bass_kernel_spmd(nc, in_maps, list(range(NCORES)), trace=trace,
                               **trace_kwargs)
    out = np.concatenate(
        [np.asarray(r["yout"]).astype(np.float32) for r in res.results], axis=1)
    return out, res


def kernel(**inputs) -> np.ndarray:
    out, _ = run(inputs)
    return out
